# revision 24
# baseline (speedup 1.0000x reference)
"""Head-sharded (tensor-parallel) causal attention block for 8 NeuronCores.

Model: B=2, S=2048, D=1024, H=16 heads (HD=64). Each core owns 2 heads
(128 features) of the QKV projections and attention, computes a partial
output projection (o_shard @ ow_shard), and the host sums the 8 partials
and adds the output bias.

Per-core kernel phases:
  1. QKV projections in transposed layout: qT/kT/vT [feat 128, seq] =
     (w_shard.T).T @ xT, accumulating over 8 K-blocks of D=1024 in PSUM.
     Bias add on VectorE while copying PSUM -> SBUF.
  2. vT -> V_aug [t 128, 16 blocks, 65] via PE transposes; column 64 = 1.0
     (ones column makes the PV matmul also produce the softmax denominator).
  3. Attention per batch: both heads processed together. Scores computed
     transposed sT[t,sq] = K@Q.T; the two heads' QK matmuls use PE row
     tiling (rows 0-63 / 64-127) and run concurrently into two PSUM banks.
     One exp on ScalarE covers both banks (scale=1/8 folded in). Causal
     mask on diagonal blocks via affine_select on GpSimd (predicate
     tp + 128k <= sf, zero-fill). PV matmuls accumulate oT_unnorm[65, sq]
     over t-blocks in PSUM (row 64 = softmax denominator). Normalize with
     reciprocal + partition-broadcast + multiply.
  4. Output projection partial[sq,1024] = oT_stack.T @ owT, two 512-wide
     matmuls into a 2-bank PSUM tile, one copy, one 512KB DMA per row block.

Matmul inputs are float32r (full-rate fp32 mode of the PE).
"""

import numpy as np

import concourse.bass as bass
import concourse.mybir as mybir
import concourse.tile as tile
from concourse import bacc
from concourse.bass import ts
from concourse.bass_utils import run_bass_kernel_spmd
from concourse.masks import make_identity

B, S, D, H = 2, 2048, 1024, 16
HD = D // H            # 64 head dim
NCORES = 8
FPC = D // NCORES      # 128 features per core
HPC = FPC // HD        # 2 heads per core
P = 128
SQ_CHUNK = 512         # query chunk (matmul free dim)
NSQ = S // SQ_CHUNK    # 4
NTB = S // P           # 16 t-blocks
DBLK = D // P          # 8 contraction blocks for projections

F32 = mybir.dt.float32
import os as _os
_MM_CHOICE = _os.environ.get("KERNEL_MM_DT", "fp16")
if _MM_CHOICE == "bf16":
    MM_DT = mybir.dt.bfloat16
    _NP_MM = "bfloat16"
elif _MM_CHOICE == "fp16":
    MM_DT = mybir.dt.float16
    _NP_MM = "float16"
else:
    MM_DT = mybir.dt.float32r
    _NP_MM = "float32"

USE_AFFINE_MASK = True

_module_cache = {}


def _build_module(repeat=1):
    nc = bacc.Bacc("TRN2", target_bir_lowering=False, debug=False)

    xT_d = nc.dram_tensor("xT", [B, D, S], MM_DT, kind="ExternalInput").ap()
    qwT_d = nc.dram_tensor("qwT", [D, FPC], MM_DT, kind="ExternalInput").ap()
    kwT_d = nc.dram_tensor("kwT", [D, FPC], MM_DT, kind="ExternalInput").ap()
    vwT_d = nc.dram_tensor("vwT", [D, FPC], MM_DT, kind="ExternalInput").ap()
    qb_d = nc.dram_tensor("qb", [FPC, 1], F32, kind="ExternalInput").ap()
    kb_d = nc.dram_tensor("kb", [FPC, 1], F32, kind="ExternalInput").ap()
    vb_d = nc.dram_tensor("vb", [FPC, 1], F32, kind="ExternalInput").ap()
    owT_d = nc.dram_tensor("owT", [FPC, D], MM_DT, kind="ExternalInput").ap()
    if not USE_AFFINE_MASK:
        mask_d = nc.dram_tensor(
            "mask", [P, NSQ, SQ_CHUNK], F32, kind="ExternalInput"
        ).ap()
    out_d = nc.dram_tensor("out", [B, S, D], MM_DT, kind="ExternalOutput").ap()

    # [B, D, S] with D split into 8 blocks of 128 partitions
    xT_r = xT_d.rearrange("b (o p) s -> b p o s", p=P)

    with tile.TileContext(nc) as tc:
        with (
            tc.tile_pool(name="singles", bufs=1) as singles,
            tc.tile_pool(name="xin", bufs=3) as xin,
            tc.tile_pool(name="ptile", bufs=5) as ptile,
            tc.tile_pool(name="small", bufs=4) as small,
            tc.tile_pool(name="outsb", bufs=3) as outsb,
        ):
            # --- constants / persistent tensors ---
            qwT_sb = singles.tile([P, DBLK, FPC], MM_DT, tag="qw")
            kwT_sb = singles.tile([P, DBLK, FPC], MM_DT, tag="kw")
            vwT_sb = singles.tile([P, DBLK, FPC], MM_DT, tag="vw")
            nc.sync.dma_start(out=qwT_sb, in_=qwT_d.rearrange("(o p) m -> p o m", p=P))
            nc.sync.dma_start(out=kwT_sb, in_=kwT_d.rearrange("(o p) m -> p o m", p=P))
            nc.sync.dma_start(out=vwT_sb, in_=vwT_d.rearrange("(o p) m -> p o m", p=P))
            qb_sb = singles.tile([FPC, 1], F32, tag="qb")
            kb_sb = singles.tile([FPC, 1], F32, tag="kb")
            vb_sb = singles.tile([FPC, 1], F32, tag="vb")
            nc.sync.dma_start(out=qb_sb, in_=qb_d)
            nc.sync.dma_start(out=kb_sb, in_=kb_d)
            nc.sync.dma_start(out=vb_sb, in_=vb_d)
            owT_sb = singles.tile([FPC, D], MM_DT, tag="ow")
            nc.sync.dma_start(out=owT_sb, in_=owT_d)
            if not USE_AFFINE_MASK:
                mask_sb = singles.tile([P, NSQ, SQ_CHUNK], F32, tag="mask")
                nc.sync.dma_start(out=mask_sb, in_=mask_d)
            ident = singles.tile([HD, HD], F32, tag="ident")
            make_identity(nc, ident)

            qT_sb = singles.tile([P, B, S], MM_DT, tag="qT")
            kT_sb = singles.tile([P, B, S], MM_DT, tag="kT")
            vT_h = [
                singles.tile([HD, B, S], F32, tag=f"vT{h}", name=f"vT{h}")
                for h in range(HPC)
            ]
            oT_sb = singles.tile([P, B, S], MM_DT, tag="oT")
            # V_aug[t, b, h, tblk, 0:64] = v features; [.., 64] = 1.0
            v_aug = singles.tile([P, B, HPC, NTB, HD + 1], MM_DT, tag="vaug")
            ones_sb = singles.tile([P, 1], F32, tag="ones")
            nc.vector.memset(ones_sb, 1.0)
            nc.vector.tensor_copy(
                out=v_aug[:, :, :, :, HD],
                in_=ones_sb[:, 0][:, None, None, None].to_broadcast([P, B, HPC, NTB]),
            )

            # ---------- repetitions (>1 only for HW timing calibration) ---
            for _rep in range(repeat):
                _emit_body(nc, tc, locals())

    return nc


def _emit_body(nc, tc, env):
    g = type("G", (), env)
    singles, xin, ptile, small, outsb = g.singles, g.xin, g.ptile, g.small, g.outsb
    qwT_sb, kwT_sb, vwT_sb = g.qwT_sb, g.kwT_sb, g.vwT_sb
    qb_sb, kb_sb, vb_sb, owT_sb, ident = g.qb_sb, g.kb_sb, g.vb_sb, g.owT_sb, g.ident
    qT_sb, kT_sb, vT_h, oT_sb, v_aug = g.qT_sb, g.kT_sb, g.vT_h, g.oT_sb, g.v_aug
    xT_r, out_d = g.xT_r, g.out_d
    mask_sb = getattr(g, "mask_sb", None)
    if True:
        if True:
            # ---------- Phase 1: QKV projections (transposed layout) ----------
            with (
                tc.tile_pool(name="ppsum", bufs=3, space="PSUM") as ppsum,
                tc.tile_pool(name="trpsum", bufs=2, space="PSUM") as trpsum,
            ):
                for b in range(B):
                    for cn in range(NSQ):
                        xt = xin.tile([P, DBLK, SQ_CHUNK], MM_DT, tag="xt")
                        for o in range(DBLK):
                            nc.sync.dma_start(
                                out=xt[:, o, :],
                                in_=xT_r[b, :, o, ts(cn, SQ_CHUNK)],
                            )
                        for wT_sb, bias_sb, kind in (
                            (qwT_sb, qb_sb, "q"),
                            (kwT_sb, kb_sb, "k"),
                            (vwT_sb, vb_sb, "v"),
                        ):
                            ps = ppsum.tile([P, SQ_CHUNK], F32, tag="proj")
                            for o in range(DBLK):
                                nc.tensor.matmul(
                                    ps,
                                    lhsT=wT_sb[:, o, :],
                                    rhs=xt[:, o, :],
                                    start=(o == 0),
                                    stop=(o == DBLK - 1),
                                )
                            if kind == "q":
                                nc.vector.tensor_scalar_add(
                                    out=qT_sb[:, b, ts(cn, SQ_CHUNK)], in0=ps,
                                    scalar1=qb_sb,
                                )
                            elif kind == "k":
                                nc.vector.tensor_scalar_add(
                                    out=kT_sb[:, b, ts(cn, SQ_CHUNK)], in0=ps,
                                    scalar1=kb_sb,
                                )
                            else:
                                for h in range(HPC):
                                    nc.vector.tensor_scalar_add(
                                        out=vT_h[h][:, b, ts(cn, SQ_CHUNK)],
                                        in0=ps[h * HD:(h + 1) * HD, :],
                                        scalar1=vb_sb[h * HD:(h + 1) * HD, :],
                                    )

                    # V_aug for this batch via PE transposes (right after its
                    # projections so attention on batch 0 can start early)
                    for h in range(HPC):
                        for j in range(NTB):
                            tp = trpsum.tile([P, HD], F32, tag="tr",
                                             name=f"tp{b}{h}{j}")
                            nc.tensor.transpose(
                                tp, in_=vT_h[h][:, b, ts(j, P)], identity=ident
                            )
                            nc.vector.tensor_copy(out=v_aug[:, b, h, j, 0:HD], in_=tp)



            # ---------- Phase 2 + 3: attention then projection, per batch ----
            with (
                tc.tile_pool(name="spsum", bufs=2, space="PSUM") as spsum,
                tc.tile_pool(name="opsum", bufs=4, space="PSUM") as opsum,
            ):
                pending = []

                def flush_norm_proj(nc):
                    b, i, po_h = pending.pop(0)
                    sq = ts(i, SQ_CHUNK)
                    for h in range(HPC):
                        hs = h * HD
                        rc = small.tile([1, SQ_CHUNK], F32, tag="rc", name=f"rc{b}{i}{h}")
                        nc.vector.reciprocal(out=rc, in_=po_h[h][HD:HD + 1, :])
                        rb = small.tile([HD, SQ_CHUNK], F32, tag="rb", name=f"rb{b}{i}{h}")
                        nc.gpsimd.partition_broadcast(out_ap=rb, in_ap=rc)
                        nc.vector.tensor_mul(
                            out=oT_sb[hs:hs + HD, b, sq],
                            in0=po_h[h][0:HD, :],
                            in1=rb,
                        )
                    for s in range(4 * i, 4 * i + 4):
                        pp = spsum.tile([P, HPC, SQ_CHUNK], F32, tag="ps",
                                        name=f"pp{b}_{s}")
                        for cc in range(2):
                            nc.tensor.matmul(
                                pp[:, cc, :],
                                lhsT=oT_sb[:, b, ts(s, P)],
                                rhs=owT_sb[:, ts(cc, SQ_CHUNK)],
                                start=True,
                                stop=True,
                            )
                        ot = outsb.tile([P, D], MM_DT, tag="ot", name=f"ot{b}_{s}")
                        nc.any.tensor_copy(
                            out=ot, in_=pp.rearrange("p a b -> p (a b)")
                        )
                        nc.sync.dma_start(out=out_d[b, ts(s, P), :], in_=ot)

                for b in range(B):
                    for i in range(NSQ):
                        sq = ts(i, SQ_CHUNK)
                        po_h = [
                            opsum.tile([HD + 1, SQ_CHUNK], F32, tag="po",
                                       name=f"po{b}_{i}_{h}")
                            for h in range(HPC)
                        ]
                        jmax = 4 * i + 3
                        for j in range(jmax + 1):
                            # Columns < 128k of diagonal blocks are fully masked;
                            # skip them in QK, exp and PV.
                            k = j - 4 * i
                            col0 = min(P * k, SQ_CHUNK - 2 * P) if k > 0 else 0
                            ps = spsum.tile([P, HPC, SQ_CHUNK], F32, tag="ps")
                            # two heads' QK in adjacent PE row-tiles (concurrent)
                            for h in range(HPC):
                                hs = h * HD
                                nc.tensor.matmul(
                                    ps[:, h, col0:],
                                    lhsT=kT_sb[hs:hs + HD, b, ts(j, P)],
                                    rhs=qT_sb[hs:hs + HD, b,
                                              i * SQ_CHUNK + col0:(i + 1) * SQ_CHUNK],
                                    start=True,
                                    stop=True,
                                )
                            pt = ptile.tile([P, HPC, SQ_CHUNK], MM_DT, tag="pt")
                            nc.scalar.activation(
                                out=pt[:, :, col0:], in_=ps[:, :, col0:],
                                func=mybir.ActivationFunctionType.Exp,
                                scale=0.125,
                            )
                            if j >= 4 * i:
                                w = P * (k + 1) - col0
                                if USE_AFFINE_MASK:
                                    # keep iff (sf - col0) - tp - (128k - col0) >= 0
                                    nc.gpsimd.affine_select(
                                        out=pt[:, :, col0:col0 + w],
                                        in_=pt[:, :, col0:col0 + w],
                                        compare_op=mybir.AluOpType.is_ge,
                                        fill=0.0,
                                        base=col0 - P * k,
                                        pattern=[[0, HPC], [1, w]],
                                        channel_multiplier=-1,
                                    )
                                else:
                                    for h in range(HPC):
                                        nc.vector.tensor_mul(
                                            out=pt[:, h, :], in0=pt[:, h, :],
                                            in1=mask_sb[:, k, :],
                                        )
                                    col0 = 0
                            for h in range(HPC):
                                nc.tensor.matmul(
                                    po_h[h][:, col0:],
                                    lhsT=v_aug[:, b, h, j, :],
                                    rhs=pt[:, h, col0:],
                                    start=(j == 0),
                                    stop=(j == jmax),
                                    skip_group_check=True,
                                )
                        # defer normalization + projection by one chunk so the
                        # next chunk's attention is emitted (and prioritized)
                        # first
                        pending.append((b, i, po_h))
                        if len(pending) > 1:
                            flush_norm_proj(nc)
                while pending:
                    flush_norm_proj(nc)


def get_module(repeat=1):
    key = ("nc", repeat)
    if key not in _module_cache:
        m = _build_module(repeat=repeat)
        m.compile()
        _module_cache[key] = m
    return _module_cache[key]


def make_in_maps(x, qw, qb, kw, kb, vw, vb, ow):
    import ml_dtypes
    mmdt = {"bfloat16": np.dtype(ml_dtypes.bfloat16),
            "float16": np.dtype(np.float16),
            "float32": np.dtype(np.float32)}[_NP_MM]
    xT = np.ascontiguousarray(x.transpose(0, 2, 1)).astype(mmdt)  # [B, D, S]
    in_maps = []
    for c in range(NCORES):
        sl = slice(c * FPC, (c + 1) * FPC)
        m = {
            "xT": xT,
            "qwT": np.ascontiguousarray(qw[sl, :].T).astype(mmdt),
            "kwT": np.ascontiguousarray(kw[sl, :].T).astype(mmdt),
            "vwT": np.ascontiguousarray(vw[sl, :].T).astype(mmdt),
            "qb": np.ascontiguousarray(qb[sl].reshape(FPC, 1)).astype(np.float32),
            "kb": np.ascontiguousarray(kb[sl].reshape(FPC, 1)).astype(np.float32),
            "vb": np.ascontiguousarray(vb[sl].reshape(FPC, 1)).astype(np.float32),
            "owT": np.ascontiguousarray(ow[:, sl].T).astype(mmdt),
        }
        if not USE_AFFINE_MASK:
            tp = np.arange(P, dtype=np.int64)[:, None, None]
            kk = np.arange(NSQ, dtype=np.int64)[None, :, None]
            sf = np.arange(SQ_CHUNK, dtype=np.int64)[None, None, :]
            m["mask"] = ((tp + P * kk) <= sf).astype(np.float32)
        in_maps.append(m)
    return in_maps


def kernel(x, qw, qb, kw, kb, vw, vb, ow, ob, _trace=False):
    x = np.asarray(x, dtype=np.float32)
    qw = np.asarray(qw, dtype=np.float32)
    qb = np.asarray(qb, dtype=np.float32)
    kw = np.asarray(kw, dtype=np.float32)
    kb = np.asarray(kb, dtype=np.float32)
    vw = np.asarray(vw, dtype=np.float32)
    vb = np.asarray(vb, dtype=np.float32)
    ow = np.asarray(ow, dtype=np.float32)
    ob = np.asarray(ob, dtype=np.float32)

    nc = get_module()
    in_maps = make_in_maps(x, qw, qb, kw, kb, vw, vb, ow)
    res = run_bass_kernel_spmd(
        nc, in_maps, core_ids=list(range(NCORES)), trace=_trace
    )
    acc = np.zeros((B, S, D), dtype=np.float64)
    for r in res.results:
        acc += r["out"].astype(np.float64)
    out = (acc + ob.astype(np.float64)).astype(np.float32)
    if _trace:
        kernel.last_results = res
    return out
